# revision 1
# baseline (speedup 1.0000x reference)
"""DecodeBox (3D YOLO-style box decode) Trainium2 Bass kernel.

Input : inp [16, 18, 48, 48, 48] f32  (= [B, A*ATTRS, D, H, W], A=3, ATTRS=6)
Output: out [16, 331776, 6] f32       (= [B, A*D*H*W, (bx,by,bz,bl,conf,cls)])

Math (per anchor a, spatial cell s=(zd,y,x), channel layout c in 0..5):
  bx = (sigmoid(v0) + gx) * 2      gx = x
  by = (sigmoid(v1) + gy) * 2      gy = y
  bz = (sigmoid(v2) + gz) * 2      gz = zd
  bl = exp(v3) * anchor_w[a]       anchor_w = (4, 8, 16)
  conf = sigmoid(v4)
  cls  = sigmoid(v5)

Sharding: batch dim across 8 cores (2 batches per core), no communication.

Per-core layout strategy: for each (b, a) block the input is [6, 110592]
channel-major while the output needs [110592, 6] attr-interleaved. Each
block is one DMA into an SBUF tile [128, 6, 864] (partition p holds spatial
positions p*864..p*864+863 of each channel); ACT computes tanh/exp (all in
one activation table set, using sigmoid(v) == 0.5*tanh(v/2)+0.5) and DVE
applies the grid/affine terms, writing into an interleaved [128, 864, 6]
tile that one contiguous DMA stores. Grid addends live in a tiny [128, 87]
constant table read through stride-0 broadcast APs. Loads are issued from
the Sync HWDGE ring and stores from the GpSimd SWDGE ring so compute-gated
stores never block later loads (keeps HBM read/write overlapped).
"""

import sys

if "/opt/trn_rl_repo" not in sys.path:
    sys.path.insert(0, "/opt/trn_rl_repo")

import numpy as np

import concourse.bacc as bacc
import concourse.bass as bass
import concourse.mybir as mybir
from concourse.bass_utils import run_bass_kernel_spmd
from concourse.tile import TileContext

B = 16
A = 3
ATTRS = 6
G = 48                # grid size per axis
S = G * G * G         # 110592 spatial positions
N_CORES = 8
B_LOC = B // N_CORES  # 2 batches per core
P = 128               # SBUF partitions
FREE = S // P         # 864 spatial positions per partition
STRIDE = 2.0          # IMG_SIZE / grid = 96 / 48
ANCHOR_W = (4.0, 8.0, 16.0)

_NC = None
last_results = None  # BassKernelResults of the most recent run (for profiling)
trace = False        # set True before calling kernel() to capture an NTFF trace


YZ = FREE // G  # 18 (y,z)-rows per partition


def _consts() -> np.ndarray:
    """[128, 87] f32 constant table, loaded once into SBUF.

    Grid addends exploit (sigmoid(v) + g)*2 == tanh(v/2) + (2g + 1) and the
    tiling s = p*864 + jj*48 + x (so x = s%48 depends only on the inner free
    index, while y/z depend only on (p, jj)); they are read through stride-0
    broadcast APs instead of materializing the full [3, S] grid.

      [:, 0:48]   2*x + 1        (same for every partition)
      [:, 48:66]  2*y + 1        per (p, jj)
      [:, 66:84]  2*z + 1        per (p, jj)
      [:, 84:87]  ln(anchor_w)
    """
    t = np.empty((P, 48 + YZ + YZ + A), dtype=np.float32)
    x = np.arange(G, dtype=np.float32)
    yz = np.arange(P, dtype=np.int64)[:, None] * YZ + np.arange(YZ)[None, :]
    t[:, 0:G] = x * STRIDE + 1.0
    t[:, G : G + YZ] = (yz % G) * STRIDE + 1.0
    t[:, G + YZ : G + 2 * YZ] = (yz // G) * STRIDE + 1.0
    t[:, G + 2 * YZ :] = np.log(np.array(ANCHOR_W, dtype=np.float32))
    return t


def _build(
    split: int = 1,
    store_engine: str = "gpsimd",
    load_engine: str = "sync",
    per_channel_loads: bool = False,
    io_bufs: int = 4,
    out_bufs: int | None = None,
    tmp_bufs: int = 6,
    sig_engine: str = "vector",
    exp_copy: bool = False,
) -> bass.Bass:
    """Build the Bass program.

    Loads are issued from the Sync engine (HWDGE ring) and stores from the
    GpSimd engine (SWDGE ring). Separate rings matter: stores are gated on
    compute semaphores, and on a shared FIFO ring a waiting store blocks
    later loads from reaching the wire, serializing reads after writes and
    losing the read/write overlap HBM can sustain (~15us on this kernel).

    split: sub-tiles per (b, a) block along the free (spatial) dim.
    """
    splits = split if isinstance(split, (list, tuple)) else [split] * (B_LOC * A)
    assert len(splits) == B_LOC * A
    for s_ in splits:
        assert FREE % s_ == 0 and (FREE // s_) % G == 0

    nc = bacc.Bacc("TRN2", target_bir_lowering=False, debug=False)
    inp = nc.dram_tensor(
        "inp", [B_LOC, A * ATTRS, G, G, G], mybir.dt.float32, kind="ExternalInput"
    )
    consts = nc.dram_tensor(
        "consts", [P, G + 2 * YZ + A], mybir.dt.float32, kind="ExternalInput"
    )
    out = nc.dram_tensor(
        "out", [B_LOC, A * S, ATTRS], mybir.dt.float32, kind="ExternalOutput"
    )

    inp_r = inp.ap().rearrange("b (a c) d h w -> (b a) c (d h w)", a=A)
    out_r = out.ap().rearrange("b (a p j) k -> (b a) p (j k)", a=A, p=P)

    F = mybir.ActivationFunctionType
    Op = mybir.AluOpType
    f32 = mybir.dt.float32

    ld = getattr(nc, load_engine)
    st = getattr(nc, store_engine)

    with TileContext(nc) as tc:
        with (
            tc.tile_pool(name="const", bufs=1) as cpool,
            tc.tile_pool(name="io", bufs=io_bufs) as iopool,
            tc.tile_pool(name="io_out", bufs=out_bufs or io_bufs) as opool,
            tc.tile_pool(name="tmp", bufs=tmp_bufs) as tpool,
        ):
            ct = cpool.tile([P, G + 2 * YZ + A], f32)
            nc.sync.dma_start(out=ct[:], in_=consts.ap())
            lw = ct[:, G + 2 * YZ :]
            sig_eng = getattr(nc, sig_engine)
            for blk in range(B_LOC * A):
                a = blk % A
                spl = splits[blk]
                FR = FREE // spl  # spatial positions per partition per sub-tile
                YZR = FR // G  # (y,z)-rows per partition per sub-tile
                blk_in = inp_r[blk].rearrange("c (p u j) -> u p c j", p=P, u=spl)
                for u in range(spl):
                    # grid addends as [P, YZR, G] stride-0 broadcast views:
                    # x varies along the inner free axis only, y/z vary per
                    # (partition, yz-row) only
                    grids = (
                        ct[:, 0:G].unsqueeze(1).broadcast_to([P, YZR, G]),
                        ct[:, G + u * YZR : G + (u + 1) * YZR]
                        .unsqueeze(2)
                        .broadcast_to([P, YZR, G]),
                        ct[:, G + YZ + u * YZR : G + YZ + (u + 1) * YZR]
                        .unsqueeze(2)
                        .broadcast_to([P, YZR, G]),
                    )
                    x = iopool.tile([P, ATTRS, FR], f32, tag="in")
                    if per_channel_loads:
                        for c in range(ATTRS):
                            ld.dma_start(out=x[:, c, :], in_=blk_in[u, :, c, :])
                    else:
                        ld.dma_start(out=x[:], in_=blk_in[u])
                    o = opool.tile([P, FR, ATTRS], f32, tag="out")
                    # All ACT ops are tanh/exp -> single exp_and_others table
                    # set for the whole kernel (sigmoid would force ~2.7us
                    # table reloads per block):
                    #   channels 0..2: sigmoid(v)*2 + 2g == tanh(v/2) + (2g+1)
                    #   channels 4,5:  sigmoid(v) == 0.5*tanh(v/2) + 0.5
                    for c in range(3):
                        t = tpool.tile([P, FR], f32, tag="t")
                        nc.scalar.activation(t[:], x[:, c, :], F.Tanh, scale=0.5)
                        nc.vector.tensor_add(
                            o[:, :, c].rearrange("p (r g) -> p r g", g=G),
                            t[:].rearrange("p (r g) -> p r g", g=G),
                            grids[c],
                        )
                    # channel 3: exp(v) * anchor_w[a] == exp(v + ln(anchor_w[a]))
                    if exp_copy:
                        # ACT pays 1.8x for strided writes; write unit-stride
                        # and let the otherwise-idle GpSimd do the interleave.
                        te = tpool.tile([P, FR], f32, tag="t")
                        nc.scalar.activation(
                            te[:], x[:, 3, :], F.Exp, bias=lw[:, a : a + 1]
                        )
                        nc.gpsimd.tensor_copy(o[:, :, 3], te[:])
                    else:
                        nc.scalar.activation(
                            o[:, :, 3], x[:, 3, :], F.Exp, bias=lw[:, a : a + 1]
                        )
                    for c in (4, 5):
                        t = tpool.tile([P, FR], f32, tag="t")
                        nc.scalar.activation(t[:], x[:, c, :], F.Tanh, scale=0.5)
                        sig_eng.tensor_scalar(
                            o[:, :, c], t[:], 0.5, 0.5, Op.mult, Op.add
                        )
                    st.dma_start(
                        out=out_r[blk][:, u * FR * ATTRS : (u + 1) * FR * ATTRS],
                        in_=o[:].rearrange("p j k -> p (j k)"),
                    )
    nc.compile()
    return nc


def kernel(inp: np.ndarray) -> np.ndarray:
    global _NC, last_results
    if _NC is None:
        _NC = _build()
    consts = _consts()
    inp = np.ascontiguousarray(np.asarray(inp), dtype=np.float32)
    assert inp.shape == (B, A * ATTRS, G, G, G), inp.shape
    in_maps = [
        {"inp": inp[i * B_LOC : (i + 1) * B_LOC], "consts": consts}
        for i in range(N_CORES)
    ]
    last_results = run_bass_kernel_spmd(
        _NC, in_maps, core_ids=list(range(N_CORES)), trace=trace
    )
    return np.concatenate([r["out"] for r in last_results.results], axis=0)

